# revision 1
# baseline (speedup 1.0000x reference)
"""DBOT Sinkhorn loss kernel for 8 Trainium2 NeuronCores.

Strategy
--------
logits_per_text == logits_per_image.T, so a single [N,N] gram matrix
S = img @ text.T serves both cross-entropy terms.  The Sinkhorn scalings
factor as P = diag(u) * P0 * diag(v) with P0 = exp(S-1), so each of the 5
iterations only needs matrix-vector products with P0 / P0^T instead of
rewriting the 256 MB matrix:

    u_A = 1 / (P0 v_A)              (row step, matrix A = P0)
    c_A = v_A * (P0^T u_A)          (col sums) -> v_A *= clamp factors
    u_B = 1 / (P0^T v_B)            (row step, matrix B = P0^T)
    c_B = v_B * (P0 u_B)            (col sums) -> v_B *= clamp factors

P0 is row-sharded across the 8 cores (1024 rows each) and kept entirely
in SBUF as bf16 (128 KB/partition), so Sinkhorn passes never touch HBM.
Column-sum style products (contract over rows) run on the tensor engine
as [K=128, M=1] mat-vecs; row-sum style products (contract over the free
axis) run on the vector engine as fused tensor_tensor_reduce against a
partition-broadcast copy of the vector.  One 32 KB AllReduce per phase
combines cross-core partials, exactly as the row-sharding requires.

The final cross entropy needs, per row i: lse_i = log sum_j exp(u_i P0_ij v_j)
(scalar-engine Exp with per-partition scale + accumulate), and the diagonal
P0_ii (computed directly from the features as exp(<img_i, text_i> - 1)).
Each core returns tiny per-row partials; the host combines them.
"""

import sys

sys.path.insert(0, "/opt/trn_rl_repo")

import numpy as np

N = 8192
D = 1024
NC = 8
R = N // NC          # rows per core
P = 128              # SBUF partitions
IB = R // P          # 8 row blocks per core
JT = N // 512        # 16 column tiles of 512
ITERS = 5
BD = 0.1 * N
BU = 0.9 * N

_BUILD_CACHE = {}


def _round_bf16(x):
    """Round-to-nearest-even fp32 -> bf16 (returned as ml_dtypes.bfloat16)."""
    from concourse import mybir

    np_bf16 = mybir.dt.np(mybir.dt.bfloat16)
    x32 = np.ascontiguousarray(x, np.float32)
    return x32.astype(np_bf16)


def _round_fp8(x):
    """Round fp32 -> fp8 e4m3 (ml_dtypes.float8_e4m3fn)."""
    from concourse import mybir

    np_f8 = mybir.dt.np(mybir.dt.float8e4)
    return np.ascontiguousarray(x, np.float32).astype(np_f8)


def _split_excess_waits(nc, max_waits=1):
    """Walrus CTRL lowering rejects instructions carrying several sem waits
    (the TileContext exit drain accumulates one per live proc).  Hoist all
    but the last wait of any multi-wait instruction into dedicated NoOps
    placed immediately before it on the same engine."""
    from concourse import mybir

    for f in nc.m.functions:
        for bb in f.blocks:
            insts = bb.instructions
            new_insts = []
            for inst in insts:
                si = inst.sync_info
                if si and si.on_wait and len(si.on_wait) > max_waits:
                    waits = list(si.on_wait)
                    head, tail = waits[:-max_waits], waits[-max_waits:]
                    for k, w in enumerate(head):
                        nop = mybir.InstNoOp(
                            name=f"{inst.name}-waitsplit-{k}",
                            engine=inst.engine,
                            ins=[],
                            outs=[],
                            sync_info=type(si)(on_wait=[w], on_update=[]),
                        )
                        new_insts.append(nop)
                    inst.sync_info = type(si)(
                        on_wait=tail, on_update=list(si.on_update or [])
                    )
                new_insts.append(inst)
            bb.instructions = new_insts


def _build():
    """Build the Bass module (same SPMD program for all 8 cores)."""
    from contextlib import ExitStack

    import concourse.bass as bass
    import concourse.tile as tile
    from concourse import mybir

    f32 = mybir.dt.float32
    bf16 = mybir.dt.bfloat16
    f8 = mybir.dt.float8e4
    AX = mybir.AxisListType
    ALU = mybir.AluOpType
    ACTF = mybir.ActivationFunctionType
    RG = [list(range(NC))]

    nc = bass.Bass("TRN2", target_bir_lowering=False, debug=False, num_devices=NC)

    # ---- external I/O ----
    imgT_d = nc.dram_tensor("imgT", [P, 8, R], f8, kind="ExternalInput")
    textT_d = nc.dram_tensor("textT", [P, JT, 8, 512], f8, kind="ExternalInput")
    textTl_d = nc.dram_tensor("textTl", [P, 8, R], f8, kind="ExternalInput")

    out_lseA = nc.dram_tensor("out_lseA", [P, IB], f32, kind="ExternalOutput")
    out_gA = nc.dram_tensor("out_gA", [P, IB], f32, kind="ExternalOutput")
    out_gB = nc.dram_tensor("out_gB", [P, IB], f32, kind="ExternalOutput")
    out_lseB = nc.dram_tensor("out_lseB", [P, 1], f32, kind="ExternalOutput")
    out_vA = nc.dram_tensor("out_vA", [P, N // P], f32, kind="ExternalOutput")
    out_uB = nc.dram_tensor("out_uB", [P, N // P], f32, kind="ExternalOutput")

    # ---- internal DRAM (collective bounce + vector staging) ----
    ccz_in = [nc.dram_tensor(f"ccz_in{i}", [N], f32) for i in range(ITERS)]
    ccz_out = [
        nc.dram_tensor(f"ccz_out{i}", [N], f32, addr_space="Shared")
        for i in range(ITERS)
    ]
    ccw_in = [nc.dram_tensor(f"ccw_in{i}", [N], f32) for i in range(ITERS)]
    ccw_out = [
        nc.dram_tensor(f"ccw_out{i}", [N], f32, addr_space="Shared")
        for i in range(ITERS)
    ]
    ccE_in = nc.dram_tensor("ccE_in", [N], f32)
    ccE_out = nc.dram_tensor("ccE_out", [N], f32, addr_space="Shared")
    d0_dram = nc.dram_tensor("d0_dram", [R], f32)
    vA16_dram = [nc.dram_tensor(f"vA16_{i}", [N], bf16) for i in range(ITERS)]
    uB16_dram = [nc.dram_tensor(f"uB16_{i}", [N], bf16) for i in range(ITERS)]

    with tile.TileContext(nc) as tc, ExitStack() as ctx:
        state = ctx.enter_context(tc.tile_pool(name="state", bufs=1))
        p0 = state.tile([P, IB, JT, 512], bf16)
        ones16 = state.tile([P, 1], bf16)
        negone = state.tile([P, 1], f32)
        y1h = state.tile([P, IB, 2], f32)
        y1 = state.tile([P, IB], f32)
        th = state.tile([P, IB, 2], f32)
        t_ = state.tile([P, IB], f32)
        uA = state.tile([P, IB], f32)
        uA16 = state.tile([P, IB], bf16)
        vB = state.tile([P, IB], f32)
        vB16 = state.tile([P, IB], bf16)
        d0 = state.tile([P, IB], f32)
        vA_blk = state.tile([P, 64], f32)
        uB_blk = state.tile([P, 64], f32)
        blk16 = state.tile([P, 64], bf16)
        wfull = state.tile([P, 64], f32)
        scA = state.tile([P, 64], f32)
        scB = state.tile([P, 64], f32)
        scC = state.tile([P, 64], f32)
        sB1 = state.tile([P, IB], f32)
        sB2 = state.tile([P, IB], f32)
        sexpAh = state.tile([P, IB, 2], f32)
        sexpA = state.tile([P, IB], f32)
        lseA_t = state.tile([P, IB], f32)
        gA_t = state.tile([P, IB], f32)
        gB_t = state.tile([P, IB], f32)

        nc.vector.memset(ones16, 1.0)
        nc.vector.memset(negone, -1.0)
        nc.vector.memset(vA_blk, 1.0)
        nc.vector.memset(vB, 1.0)
        nc.vector.memset(vB16, 1.0)

        # ============ feature load + diag pre-phase ============
        feat_ctx = ExitStack()
        featp = feat_ctx.enter_context(tc.tile_pool(name="featp", bufs=1))
        imgT_sb = featp.tile([P, 8, R], f8)
        nc.sync.dma_start(out=imgT_sb[:], in_=imgT_d.ap())

        with (
            tc.tile_pool(name="prep", bufs=1) as prep,
            tc.tile_pool(name="preps", bufs=1, space="PSUM") as preps,
        ):
            ttl = prep.tile([P, 8, R], f8)
            nc.sync.dma_start(out=ttl[:], in_=textTl_d.ap())
            prodD = prep.tile([P, 8, R], bf16)
            nc.vector.tensor_mul(prodD[:], imgT_sb[:], ttl[:])
            ps_d = preps.tile([1, 2, 512], f32)
            for h in range(2):
                for db in range(8):
                    nc.tensor.matmul(
                        ps_d[0:1, h, :],
                        ones16[:],
                        prodD[:, db, h * 512 : (h + 1) * 512],
                        start=(db == 0),
                        stop=(db == 7),
                    )
            sd = prep.tile([1, R], f32)
            nc.scalar.activation(
                sd[0:1, :], ps_d[0:1, :, :], ACTF.Exp, bias=negone[0:1, :]
            )
            nc.sync.dma_start(out=d0_dram.ap(), in_=sd[0:1, :])
        nc.gpsimd.dma_start(
            out=d0[:], in_=d0_dram.ap().rearrange("(ib p) -> p ib", p=P)
        )

        # ============ M phase: S = img@text.T, P0 = exp(S-1) ============
        # fused: y1 partials (row sums, via ACT accumulate) and z partials
        # (col sums, via ones mat-vec) for iteration 0 (v_A = v_B = 1).
        y1acc = state.tile([P, IB, JT], f32)
        mm_ctx = ExitStack()
        mp = mm_ctx.enter_context(tc.tile_pool(name="mp", bufs=2))
        mps = mm_ctx.enter_context(tc.tile_pool(name="mps", bufs=2, space="PSUM"))
        mzs = mm_ctx.enter_context(tc.tile_pool(name="mzs", bufs=1, space="PSUM"))
        msc = mm_ctx.enter_context(tc.tile_pool(name="msc", bufs=2))
        for js in range(8):  # slabs of 2 j-tiles
            tbuf = mp.tile([P, 2, 8, 512], f8, tag="textT")
            nc.sync.dma_start(out=tbuf[:], in_=textT_d.ap()[:, js * 2 : js * 2 + 2, :, :])
            zps = mzs.tile([1, 2, 512], f32, tag="zps")
            for ib in range(IB):
                sps = mps.tile([P, 2, 512], f32, tag="sps")
                for db in range(4):
                    for jl in range(2):
                        nc.tensor.matmul(
                            sps[:, jl, :],
                            imgT_sb[:, db * 2 : db * 2 + 2, ib * P : (ib + 1) * P],
                            tbuf[:, jl, db * 2 : db * 2 + 2, :],
                            start=(db == 0),
                            stop=(db == 3),
                            perf_mode=mybir.MatmulPerfMode.DoubleRow,
                        )
                for jl in range(2):
                    jt = js * 2 + jl
                    nc.scalar.activation(
                        p0[:, ib, jt, :],
                        sps[:, jl, :],
                        ACTF.Exp,
                        bias=negone[:],
                        accum_out=y1acc[:, ib, jt : jt + 1],
                    )
                    nc.tensor.matmul(
                        zps[0:1, jl, :],
                        ones16[:],
                        p0[:, ib, jt, :],
                        start=(ib == 0),
                        stop=(ib == IB - 1),
                    )
            zrow = msc.tile([1, 2, 512], f32, tag="zrow")
            nc.scalar.copy(zrow[:], zps[:])
            nc.sync.dma_start(
                out=ccz_in[0].ap()[js * 1024 : (js + 1) * 1024], in_=zrow[0:1, :, :]
            )
        mm_ctx.close()
        feat_ctx.close()
        nc.vector.reduce_sum(y1[:], y1acc[:], axis=AX.X)

        # ============ post-M pools ============
        bcp = ctx.enter_context(tc.tile_pool(name="bcp", bufs=1))
        vA_bc = bcp.tile([P, N], bf16)
        uB_bc = bcp.tile([P, N], bf16)
        scrA_p = ctx.enter_context(tc.tile_pool(name="scrA", bufs=2))
        scrB_p = ctx.enter_context(tc.tile_pool(name="scrB", bufs=2))
        sk_ps_ctx = ExitStack()
        sk_ps = sk_ps_ctx.enter_context(tc.tile_pool(name="sk_ps", bufs=4, space="PSUM"))
        sk_sc = ctx.enter_context(tc.tile_pool(name="sk_sc", bufs=2))

        def halfview(tile_ap, ib, h):
            # [P, 4096] view of p0 row-block ib, half h
            return tile_ap[:, ib, h * 8 : (h + 1) * 8, :].rearrange("p a b -> p (a b)")

        def pe_colsum(lhs_vec16, cc_dst):
            """w_j = sum_i lhs_i * P0_ij  (per-core partial), DMA'd to cc_dst."""
            for jt in range(JT):
                ps = sk_ps.tile([1, 512], f32, tag="skps")
                for ib in range(IB):
                    nc.tensor.matmul(
                        ps[0:1, :],
                        lhs_vec16[:, ib : ib + 1],
                        p0[:, ib, jt, :],
                        start=(ib == 0),
                        stop=(ib == IB - 1),
                    )
                row = sk_sc.tile([1, 512], f32, tag="skrow")
                nc.scalar.copy(row[:], ps[:])
                nc.sync.dma_start(
                    out=cc_dst.ap()[jt * 512 : (jt + 1) * 512], in_=row[0:1, :]
                )

        def dve_rowsum(bc_tile, acc_h, acc):
            """y_i = sum_j P0_ij * bc_j.  Split across engines: a few units
            use the fused DVE scalar_tensor_tensor (1x mode), the rest use a
            2x-mode DVE multiply plus a ScalarE copy-accumulate, balancing
            DVE against ACT."""
            for ib in range(IB):
                for h in range(2):
                    u = ib * 2 + h
                    if u % 2 == 0:
                        scr = scrA_p.tile([P, N // 2], bf16, tag="ttr_out")
                        nc.vector.scalar_tensor_tensor(
                            out=scr[:],
                            in0=halfview(p0, ib, h),
                            scalar=1.0,
                            in1=bc_tile[:, h * (N // 2) : (h + 1) * (N // 2)],
                            op0=ALU.mult,
                            op1=ALU.mult,
                            accum_out=acc_h[:, ib, h : h + 1],
                        )
                    else:
                        scr = scrA_p.tile([P, N // 2], bf16, tag="ttr_out")
                        nc.vector.tensor_mul(
                            scr[:],
                            halfview(p0, ib, h),
                            bc_tile[:, h * (N // 2) : (h + 1) * (N // 2)],
                        )
                        scr2 = scrB_p.tile([P, N // 2], bf16, tag="exp_out")
                        nc.scalar.activation(
                            scr2[:], scr[:], ACTF.Copy,
                            accum_out=acc_h[:, ib, h : h + 1],
                        )
            nc.vector.reduce_sum(acc[:], acc_h[:], axis=AX.X)

        def colstep(vec, c, s1, s2, s3):
            """vec *= max(BD/c, 1) * min(BU/(c*max(BD/c,1)), 1)   (in place).
            s1..s3 are scratch tiles shaped like vec."""
            nc.vector.reciprocal(s1[:], c[:])
            nc.vector.tensor_scalar(s1[:], s1[:], BD, 1.0, op0=ALU.mult, op1=ALU.max)
            nc.vector.tensor_mul(s2[:], c[:], s1[:])  # c * f1
            nc.vector.tensor_mul(vec[:], vec[:], s1[:])
            nc.vector.reciprocal(s3[:], s2[:])
            nc.vector.tensor_scalar(s3[:], s3[:], BU, 1.0, op0=ALU.mult, op1=ALU.min)
            nc.vector.tensor_mul(vec[:], vec[:], s3[:])

        blkview = lambda t: t.ap().rearrange("(p q) -> p q", p=P)
        bcast = lambda t: bass.AP(tensor=t.ap().tensor, offset=0, ap=[[0, P], [1, N]])

        # ============ Sinkhorn iterations ============
        for it in range(ITERS):
            if it > 0:
                dve_rowsum(vA_bc, y1h, y1)  # y1 = P0 v_A   (local rows)
                pe_colsum(vB16, ccz_in[it])  # z partial = P0^T v_B
            nc.gpsimd.collective_compute(
                "AllReduce", ALU.add, replica_groups=RG,
                ins=[ccz_in[it].ap()], outs=[ccz_out[it].ap()],
            )
            # row step A: u_A = 1 / y1
            nc.vector.reciprocal(uA[:], y1[:])
            nc.vector.tensor_copy(uA16[:], uA[:])
            # row step B: u_B = 1 / z  (full vector, replicated on every core)
            nc.sync.dma_start(out=wfull[:], in_=blkview(ccz_out[it]))
            nc.vector.reciprocal(uB_blk[:], wfull[:])
            nc.vector.tensor_copy(blk16[:], uB_blk[:])
            nc.sync.dma_start(out=blkview(uB16_dram[it]), in_=blk16[:])
            nc.gpsimd.dma_start(
                out=uB_bc[:], in_=bcast(uB16_dram[it])
            )

            # phase 2
            pe_colsum(uA16, ccw_in[it])  # w partial = P0^T u_A
            dve_rowsum(uB_bc, th, t_)  # t = P0 u_B  (local rows)
            nc.gpsimd.collective_compute(
                "AllReduce", ALU.add, replica_groups=RG,
                ins=[ccw_in[it].ap()], outs=[ccw_out[it].ap()],
            )
            # col step B (local): c_B = vB * t
            nc.vector.tensor_mul(sB1[:], vB[:], t_[:])
            colstep(vB, sB1, sB2, gA_t, gB_t)  # reuse gA_t/gB_t as scratch here
            nc.vector.tensor_copy(vB16[:], vB[:])
            # col step A (full): c_A = vA * w
            nc.sync.dma_start(out=wfull[:], in_=blkview(ccw_out[it]))
            nc.vector.tensor_mul(scA[:], vA_blk[:], wfull[:])
            colstep(vA_blk, scA, scB, scC, wfull)
            nc.vector.tensor_copy(blk16[:], vA_blk[:])
            nc.sync.dma_start(out=blkview(vA16_dram[it]), in_=blk16[:])
            nc.gpsimd.dma_start(
                out=vA_bc[:], in_=bcast(vA16_dram[it])
            )

        sk_ps_ctx.close()

        # ============ cross entropy ============
        # CE-A: lse_i = log sum_j exp(u_i * P0_ij * vA_j)
        for ib in range(IB):
            for h in range(2):
                scr = scrA_p.tile([P, N // 2], bf16, tag="ttr_out")
                nc.vector.tensor_mul(
                    scr[:], halfview(p0, ib, h),
                    vA_bc[:, h * (N // 2) : (h + 1) * (N // 2)],
                )
                scre = scrB_p.tile([P, N // 2], bf16, tag="exp_out")
                nc.scalar.activation(
                    scre[:], scr[:], ACTF.Exp,
                    scale=uA[:, ib : ib + 1],
                    accum_out=sexpAh[:, ib, h : h + 1],
                )
        nc.vector.reduce_sum(sexpA[:], sexpAh[:], axis=AX.X)
        nc.scalar.activation(lseA_t[:], sexpA[:], ACTF.Ln)
        nc.sync.dma_start(out=out_lseA.ap(), in_=lseA_t[:])

        # CE-B: partial over local rows of sum_i exp(vB_i * P0_ij * uB_j)
        with tc.tile_pool(name="ce_ps", bufs=1, space="PSUM") as cepsp:
            for h in range(2):
                ceps = cepsp.tile([1, 8, 512], f32, tag="ceps")
                for ib in range(IB):
                    scr = scrA_p.tile([P, N // 2], bf16, tag="ttr_out")
                    nc.vector.tensor_mul(
                        scr[:], halfview(p0, ib, h),
                        uB_bc[:, h * (N // 2) : (h + 1) * (N // 2)],
                    )
                    scre = scrB_p.tile([P, N // 2], bf16, tag="exp_out")
                    nc.scalar.activation(
                        scre[:], scr[:], ACTF.Exp, scale=vB[:, ib : ib + 1]
                    )
                    for j8 in range(8):
                        nc.tensor.matmul(
                            ceps[0:1, j8, :],
                            ones16[:],
                            scre[:, j8 * 512 : (j8 + 1) * 512],
                            start=(ib == 0),
                            stop=(ib == IB - 1),
                        )
                for j8 in range(8):
                    cerow = sk_sc.tile([1, 512], f32, tag="skrow")
                    nc.scalar.copy(cerow[:], ceps[0:1, j8, :])
                    off = h * (N // 2) + j8 * 512
                    nc.sync.dma_start(
                        out=ccE_in.ap()[off : off + 512], in_=cerow[0:1, :]
                    )
        nc.gpsimd.collective_compute(
            "AllReduce", ALU.add, replica_groups=RG,
            ins=[ccE_in.ap()], outs=[ccE_out.ap()],
        )
        nc.sync.dma_start(out=wfull[:], in_=blkview(ccE_out))
        nc.scalar.activation(scA[:], wfull[:], ACTF.Ln)  # lseB block
        lseBs = state.tile([P, 1], f32)
        nc.vector.reduce_sum(lseBs[:], scA[:], axis=AX.X)
        nc.sync.dma_start(out=out_lseB.ap(), in_=lseBs[:])

        # diag factors and final vectors
        nc.vector.tensor_mul(gA_t[:], uA[:], d0[:])
        nc.sync.dma_start(out=out_gA.ap(), in_=gA_t[:])
        nc.vector.tensor_mul(gB_t[:], vB[:], d0[:])
        nc.sync.dma_start(out=out_gB.ap(), in_=gB_t[:])
        nc.sync.dma_start(out=out_vA.ap(), in_=vA_blk[:])
        nc.sync.dma_start(out=out_uB.ap(), in_=uB_blk[:])

    _split_excess_waits(nc)
    return nc


def _get_nc():
    if "nc" not in _BUILD_CACHE:
        _BUILD_CACHE["nc"] = _build()
    return _BUILD_CACHE["nc"]


def _fallback(img, txt, labels):
    """Reference math on host (only for unexpected label patterns)."""
    S = img.astype(np.float64) @ txt.astype(np.float64).T

    def sink(Pin):
        n = Pin.shape[0]
        Pm = np.exp(-Pin)
        for _ in range(ITERS):
            Pm = (1.0 / Pm.sum(1))[:, None] * Pm
            Pm = Pm * np.maximum(BD / Pm.sum(0), 1.0)[None, :]
            Pm = Pm * np.minimum(BU / Pm.sum(0), 1.0)[None, :]
        return Pm

    def ce(logits, lab):
        m = logits.max(1, keepdims=True)
        lse = np.log(np.exp(logits - m).sum(1)) + m[:, 0]
        picked = logits[np.arange(logits.shape[0]), lab]
        return np.mean(lse - picked)

    lab = np.asarray(labels, np.int64)
    loss = 0.5 * (ce(sink(1.0 - S), lab) + ce(sink(1.0 - S.T), lab))
    return np.float32(loss)


def kernel(all_image_features, all_text_features, logit_scale, labels):
    from concourse.bass_utils import run_bass_kernel_spmd

    img = np.ascontiguousarray(np.asarray(all_image_features), np.float32)
    txt = np.ascontiguousarray(np.asarray(all_text_features), np.float32)
    lab = np.asarray(labels)
    assert img.shape == (N, D) and txt.shape == (N, D)
    if not np.array_equal(lab.astype(np.int64), np.arange(N, dtype=np.int64)):
        return _fallback(img, txt, lab)

    img8 = _round_fp8(img)
    txt8 = _round_fp8(txt)

    # DoubleRow layout: dim g = db*2 + c maps to d = db*256 + c*128 + p,
    # i.e. features reshaped [ .., 4(db), 2(c), 128(p)] on the d axis.
    # textT[p, jt, g, j] = txt[jt*512 + j, d(g, p)]
    textT = np.ascontiguousarray(
        txt8.reshape(JT, 512, 4, 2, P).transpose(4, 0, 2, 3, 1).reshape(P, JT, 8, 512)
    )
    in_maps = []
    for k in range(NC):
        sl = slice(k * R, (k + 1) * R)
        imgT = np.ascontiguousarray(
            img8[sl].reshape(R, 4, 2, P).transpose(3, 1, 2, 0).reshape(P, 8, R)
        )
        textTl = np.ascontiguousarray(
            txt8[sl].reshape(R, 4, 2, P).transpose(3, 1, 2, 0).reshape(P, 8, R)
        )
        in_maps.append({"imgT": imgT, "textT": textT, "textTl": textTl})

    nc = _get_nc()
    _BUILD_CACHE["in_maps"] = in_maps
    res = run_bass_kernel_spmd(nc, in_maps, list(range(NC)))

    # ---- host-side unshard / combine (O(N) work) ----
    r0 = res.results[0]
    vA = r0["out_vA"].astype(np.float64).reshape(N)
    uB = r0["out_uB"].astype(np.float64).reshape(N)
    lseB_sum = r0["out_lseB"].astype(np.float64).sum()

    lseA_sum = 0.0
    diagA_sum = 0.0
    diagB_sum = 0.0
    for k in range(NC):
        rk = res.results[k]
        # [p, ib] -> local row i = ib*128 + p
        lseA_sum += rk["out_lseA"].astype(np.float64).sum()
        gA = rk["out_gA"].astype(np.float64).T.reshape(R)  # gA[i] = uA_i * P0_ii
        gB = rk["out_gB"].astype(np.float64).T.reshape(R)  # gB[i] = vB_i * P0_ii
        sl = slice(k * R, (k + 1) * R)
        diagA_sum += float(gA @ vA[sl])
        diagB_sum += float(gB @ uB[sl])

    lossA = (lseA_sum - diagA_sum) / N
    lossB = (lseB_sum - diagB_sum) / N
    return np.float32(0.5 * (lossA + lossB))



# revision 2
# speedup vs baseline: 1.6621x; 1.6621x over previous
"""DBOT Sinkhorn loss kernel for 8 Trainium2 NeuronCores.

Strategy (v2 — all-PE dual-slab)
--------------------------------
S = img @ text.T serves both cross-entropy terms (logits_per_text = S.T).
The Sinkhorn scalings factor as P = diag(u) P0 diag(v), P0 = exp(S-1), so
each iteration only needs matrix-vector products with P0 / P0^T.

Each core k holds TWO fp8 slabs in SBUF (64 KB/partition each):
  slabA = P0[rows Ik, :]      (row block of P0)
  slabB = P0^T[rows Ik, :]    (= column block of P0, transposed)
built by two fp8-DoubleRow gram matmuls (textT / imgT as moving operands).
With both slabs, every Sinkhorn product becomes a PE column-sum mat-vec
(contract over partitions) in fp8 DoubleRow — ~2x the bf16 rate — and the
tensor engine never idles long enough for the HAM clock-gate to rethrottle.

Each product yields an [N] partial; a ReduceScatter hands core k exactly
its local slice, which is also the slice needed for the next stationary
vector.  Stationary vectors are rescaled by their first element and cast
to fp8 (the Sinkhorn recurrence is self-correcting; validated to ~5e-6
final relative error in simulation).

The cross entropy uses that final-plan entries are small (~0.1):
  lse_i = log(N + r_i + r_i^2/2N + r_i^3/6N^2 + (e^d - 1 - d - d^2/2 - d^3/6))
with r_i = u_i (P0 v)_i (one more product) and d = u_i P0_ii v_i (diag from
features).  Host combines per-core [R] outputs in float64.
"""

import sys

sys.path.insert(0, "/opt/trn_rl_repo")

import numpy as np

N = 8192
D = 1024
NC = 8
R = N // NC          # rows per core
P = 128              # SBUF partitions
IB = R // P          # 8 row blocks per core
JT = N // 512        # 16 column tiles of 512
ITERS = 5
BD = 0.1 * N
BU = 0.9 * N
NPROD = 2 + 2 + 4 * (ITERS - 1) + 2   # products: it0 (y,w per chain) + 4/iter + final

_BUILD_CACHE = {}


def _round_fp8(x):
    from concourse import mybir

    np_f8 = mybir.dt.np(mybir.dt.float8e4)
    return np.ascontiguousarray(x, np.float32).astype(np_f8)


def _split_excess_waits(nc, max_waits=1):
    """Walrus CTRL lowering rejects instructions carrying several sem waits.
    Hoist all but the last wait of any multi-wait instruction into dedicated
    NoOps placed immediately before it on the same engine."""
    from concourse import mybir

    for f in nc.m.functions:
        for bb in f.blocks:
            insts = bb.instructions
            new_insts = []
            for inst in insts:
                si = inst.sync_info
                if si and si.on_wait and len(si.on_wait) > max_waits:
                    waits = list(si.on_wait)
                    head, tail = waits[:-max_waits], waits[-max_waits:]
                    for k, w in enumerate(head):
                        nop = mybir.InstNoOp(
                            name=f"{inst.name}-waitsplit-{k}",
                            engine=inst.engine,
                            ins=[],
                            outs=[],
                            sync_info=type(si)(on_wait=[w], on_update=[]),
                        )
                        new_insts.append(nop)
                    inst.sync_info = type(si)(
                        on_wait=tail, on_update=list(si.on_update or [])
                    )
                new_insts.append(inst)
            bb.instructions = new_insts


def _build():
    """Build the Bass module (same SPMD program for all 8 cores)."""
    from contextlib import ExitStack

    import concourse.bass as bass
    import concourse.tile as tile
    from concourse import mybir

    f32 = mybir.dt.float32
    bf16 = mybir.dt.bfloat16
    f8 = mybir.dt.float8e4
    ALU = mybir.AluOpType
    ACTF = mybir.ActivationFunctionType
    DR = mybir.MatmulPerfMode.DoubleRow
    RG = [list(range(NC))]

    nc = bass.Bass("TRN2", target_bir_lowering=False, debug=False, num_devices=NC)

    # ---- external I/O ----
    imgT_l = nc.dram_tensor("imgT_l", [P, 8, R], f8, kind="ExternalInput")
    textT_l = nc.dram_tensor("textT_l", [P, 8, R], f8, kind="ExternalInput")
    textT_g = nc.dram_tensor("textT_g", [P, JT, 8, 512], f8, kind="ExternalInput")
    imgT_g = nc.dram_tensor("imgT_g", [P, JT, 8, 512], f8, kind="ExternalInput")

    out_rA = nc.dram_tensor("out_rA", [P, IB], f32, kind="ExternalOutput")
    out_rB = nc.dram_tensor("out_rB", [P, IB], f32, kind="ExternalOutput")
    out_dA = nc.dram_tensor("out_dA", [P, IB], f32, kind="ExternalOutput")
    out_dB = nc.dram_tensor("out_dB", [P, IB], f32, kind="ExternalOutput")

    # ---- internal DRAM ----
    d0_dram = nc.dram_tensor("d0_dram", [R], f32)
    ps_in = [nc.dram_tensor(f"ps_in{t}", [N], f32) for t in range(NPROD)]
    ps_out = [nc.dram_tensor(f"ps_out{t}", [R], f32) for t in range(NPROD)]
    v0_d = [nc.dram_tensor(f"v0_d{t}", [1], f32) for t in range(2 * ITERS)]

    def bcast1(dram_t):
        ap = dram_t.ap()
        return bass.AP(tensor=ap.tensor, offset=0, ap=[[0, P], [1, 1]])

    with tile.TileContext(nc) as tc, ExitStack() as ctx:
        state = ctx.enter_context(tc.tile_pool(name="state", bufs=1))
        slabA = state.tile([P, IB, JT, 512], f8)
        slabB = state.tile([P, IB, JT, 512], f8)
        statA = state.tile([P, IB, 16], f8)
        statB = state.tile([P, IB, 16], f8)
        sA = state.tile([1, 1], f32)
        sB = state.tile([1, 1], f32)
        ones16 = state.tile([P, 1], bf16)
        negone = state.tile([P, 1], f32)
        d0 = state.tile([P, IB], f32)
        uA = state.tile([P, IB], f32)
        uB = state.tile([P, IB], f32)
        vA = state.tile([P, IB], f32)
        vB = state.tile([P, IB], f32)
        yA = state.tile([P, IB], f32)
        yB = state.tile([P, IB], f32)
        wA = state.tile([P, IB], f32)
        wB = state.tile([P, IB], f32)
        c1 = state.tile([P, IB], f32)
        c2 = state.tile([P, IB], f32)
        c3 = state.tile([P, IB], f32)
        y0A = state.tile([P, 1], f32)
        y0B = state.tile([P, 1], f32)
        v0A = state.tile([P, 1], f32)
        v0B = state.tile([P, 1], f32)
        rvA = state.tile([P, 1], f32)
        rvB = state.tile([P, 1], f32)

        nc.vector.memset(ones16, 1.0)
        nc.vector.memset(negone, -1.0)
        nc.vector.memset(statA, 1.0)
        nc.vector.memset(statB, 1.0)
        nc.vector.memset(sA, 1.0)
        nc.vector.memset(sB, 1.0)
        nc.vector.memset(vA, 1.0)
        nc.vector.memset(vB, 1.0)

        # ============ feature load + diag pre-phase ============
        feat_ctx = ExitStack()
        featp = feat_ctx.enter_context(tc.tile_pool(name="featp", bufs=1))
        imgT_sb = featp.tile([P, 8, R], f8)
        textTl_sb = featp.tile([P, 8, R], f8)
        nc.sync.dma_start(out=imgT_sb[:], in_=imgT_l.ap())
        nc.sync.dma_start(out=textTl_sb[:], in_=textT_l.ap())

        with (
            tc.tile_pool(name="prep", bufs=1) as prep,
            tc.tile_pool(name="preps", bufs=1, space="PSUM") as preps,
        ):
            prodD = prep.tile([P, 8, R], bf16)
            nc.vector.tensor_mul(prodD[:], imgT_sb[:], textTl_sb[:])
            ps_d = preps.tile([1, 2, 512], f32)
            for h in range(2):
                for db in range(8):
                    nc.tensor.matmul(
                        ps_d[0:1, h, :],
                        ones16[:],
                        prodD[:, db, h * 512 : (h + 1) * 512],
                        start=(db == 0),
                        stop=(db == 7),
                    )
            sd = prep.tile([1, R], f32)
            nc.scalar.activation(
                sd[0:1, :], ps_d[0:1, :, :], ACTF.Exp, bias=negone[0:1, :]
            )
            nc.sync.dma_start(out=d0_dram.ap(), in_=sd[0:1, :])
        nc.gpsimd.dma_start(
            out=d0[:], in_=d0_dram.ap().rearrange("(ib p) -> p ib", p=P)
        )

        # ============ gram phase: slabA = exp(S-1), slabB = exp(S.T-1) ============
        gram_ctx = ExitStack()
        mvp = gram_ctx.enter_context(tc.tile_pool(name="mvp", bufs=2))
        gps = gram_ctx.enter_context(tc.tile_pool(name="gps", bufs=2, space="PSUM"))
        for stat_sb, mv_d, slab in ((imgT_sb, textT_g, slabA), (textTl_sb, imgT_g, slabB)):
            for jc in range(4):
                mv = mvp.tile([P, 4, 8, 512], f8, tag="mv")
                nc.sync.dma_start(out=mv[:], in_=mv_d.ap()[:, jc * 4 : (jc + 1) * 4, :, :])
                for ib in range(8):
                    ps = gps.tile([P, 4, 512], f32, tag="gps")
                    for jl in range(4):
                        for db in range(4):
                            nc.tensor.matmul(
                                ps[:, jl, :],
                                stat_sb[:, db * 2 : db * 2 + 2, ib * P : (ib + 1) * P],
                                mv[:, jl, db * 2 : db * 2 + 2, :],
                                start=(db == 0),
                                stop=(db == 3),
                                perf_mode=DR,
                            )
                    nc.scalar.activation(
                        slab[:, ib, jc * 4 : (jc + 1) * 4, :],
                        ps[:],
                        ACTF.Exp,
                        bias=negone[:],
                    )
        gram_ctx.close()
        feat_ctx.close()

        # ============ product machinery ============
        prod_ctx = ExitStack()
        pps = prod_ctx.enter_context(tc.tile_pool(name="pps", bufs=4, space="PSUM"))
        stgp = prod_ctx.enter_context(tc.tile_pool(name="stgp", bufs=2))

        def product(slab, stat, s_tile, t):
            """ps_out[t] <- RS over cores of [N]-partial of (stat . slab)."""
            for jh in range(4):
                stg = stgp.tile([1, 4, 512], f32, tag="stg")
                for jl in range(4):
                    jt = jh * 4 + jl
                    ps = pps.tile([1, 512], f32, tag="pps")
                    for q in range(4):
                        nc.tensor.matmul(
                            ps[0:1, :],
                            stat[:, 2 * q : 2 * q + 2, 0:1],
                            slab[:, 2 * q : 2 * q + 2, jt, :],
                            start=(q == 0),
                            stop=(q == 3),
                            perf_mode=DR,
                        )
                    nc.scalar.activation(
                        stg[0:1, jl, :], ps[0:1, :], ACTF.Copy, scale=s_tile[0:1, :]
                    )
                nc.sync.dma_start(
                    out=ps_in[t].ap()[jh * 2048 : (jh + 1) * 2048], in_=stg[0:1, :, :]
                )
            nc.gpsimd.collective_compute(
                "ReduceScatter", ALU.add, replica_groups=RG,
                ins=[ps_in[t].ap()], outs=[ps_out[t].ap()],
            )

        def recv_y(t, y, u, y0, stat, s_tile):
            """u = 1/y; stat = f8(u * y[0]); s = 1/y[0] (partial rescale)."""
            nc.sync.dma_start(out=y[:], in_=ps_out[t].ap().rearrange("(ib p) -> p ib", p=P))
            nc.gpsimd.dma_start(out=y0[:], in_=bcast1(ps_out[t]))
            nc.vector.reciprocal(u[:], y[:])
            nc.scalar.activation(stat[:, :, 0:1], u[:], ACTF.Copy, scale=y0[:])
            nc.vector.reciprocal(s_tile[:], y0[0:1, 0:1])

        def recv_w(t, w, v):
            """colstep: v *= max(BD/c,1)*min(BU/(c*f1),1), c = v.w."""
            nc.sync.dma_start(out=w[:], in_=ps_out[t].ap().rearrange("(ib p) -> p ib", p=P))
            nc.vector.tensor_mul(c1[:], v[:], w[:])
            nc.vector.reciprocal(c2[:], c1[:])
            nc.vector.tensor_scalar(c2[:], c2[:], BD, 1.0, op0=ALU.mult, op1=ALU.max)
            nc.vector.tensor_mul(c3[:], c1[:], c2[:])
            nc.vector.tensor_mul(v[:], v[:], c2[:])
            nc.vector.reciprocal(c1[:], c3[:])
            nc.vector.tensor_scalar(c1[:], c1[:], BU, 1.0, op0=ALU.mult, op1=ALU.min)
            nc.vector.tensor_mul(v[:], v[:], c1[:])

        def quant_v(v, vd, v0, rv, stat, s_tile):
            """stat = f8(v / v[0]); s = v[0] (partial rescale)."""
            nc.sync.dma_start(out=vd.ap().rearrange("(a q) -> a q", a=1), in_=v[0:1, 0:1])
            nc.gpsimd.dma_start(out=v0[:], in_=bcast1(vd))
            nc.vector.reciprocal(rv[:], v0[:])
            nc.scalar.activation(stat[:, :, 0:1], v[:], ACTF.Copy, scale=rv[:])
            nc.scalar.copy(s_tile[:], v0[0:1, 0:1])

        # ============ Sinkhorn iterations ============
        t = 0
        for it in range(ITERS):
            # y products (it0 uses the memset ones stationaries, s=1)
            tA, tB = t, t + 1
            product(slabB, statA, sA, tA)
            product(slabA, statB, sB, tB)
            recv_y(tA, yA, uA, y0A, statA, sA)
            recv_y(tB, yB, uB, y0B, statB, sB)
            # w products with u stationaries
            tA, tB = t + 2, t + 3
            product(slabA, statA, sA, tA)
            product(slabB, statB, sB, tB)
            recv_w(tA, wA, vA)
            quant_v(vA, v0_d[2 * it], v0A, rvA, statA, sA)
            recv_w(tB, wB, vB)
            quant_v(vB, v0_d[2 * it + 1], v0B, rvB, statB, sB)
            t += 4

        # final row-sum products for the cross entropy
        tA, tB = t, t + 1
        product(slabB, statA, sA, tA)
        product(slabA, statB, sB, tB)
        nc.sync.dma_start(out=yA[:], in_=ps_out[tA].ap().rearrange("(ib p) -> p ib", p=P))
        nc.sync.dma_start(out=yB[:], in_=ps_out[tB].ap().rearrange("(ib p) -> p ib", p=P))
        prod_ctx.close()

        # outputs: r = u.y6, d = u.d0.v
        nc.vector.tensor_mul(c1[:], uA[:], yA[:])
        nc.sync.dma_start(out=out_rA.ap(), in_=c1[:])
        nc.vector.tensor_mul(c2[:], uB[:], yB[:])
        nc.sync.dma_start(out=out_rB.ap(), in_=c2[:])
        nc.vector.tensor_mul(c3[:], uA[:], d0[:])
        nc.vector.tensor_mul(c3[:], c3[:], vA[:])
        nc.sync.dma_start(out=out_dA.ap(), in_=c3[:])
        nc.vector.tensor_mul(wA[:], uB[:], d0[:])
        nc.vector.tensor_mul(wA[:], wA[:], vB[:])
        nc.sync.dma_start(out=out_dB.ap(), in_=wA[:])

    _split_excess_waits(nc)
    return nc


def _get_nc():
    if "nc" not in _BUILD_CACHE:
        _BUILD_CACHE["nc"] = _build()
    return _BUILD_CACHE["nc"]


def _fallback(img, txt, labels):
    """Reference math on host (only for unexpected label patterns)."""
    S = img.astype(np.float64) @ txt.astype(np.float64).T

    def sink(Pin):
        n = Pin.shape[0]
        Pm = np.exp(-Pin)
        for _ in range(ITERS):
            Pm = (1.0 / Pm.sum(1))[:, None] * Pm
            Pm = Pm * np.maximum(BD / Pm.sum(0), 1.0)[None, :]
            Pm = Pm * np.minimum(BU / Pm.sum(0), 1.0)[None, :]
        return Pm

    def ce(logits, lab):
        m = logits.max(1, keepdims=True)
        lse = np.log(np.exp(logits - m).sum(1)) + m[:, 0]
        picked = logits[np.arange(logits.shape[0]), lab]
        return np.mean(lse - picked)

    lab = np.asarray(labels, np.int64)
    loss = 0.5 * (ce(sink(1.0 - S), lab) + ce(sink(1.0 - S.T), lab))
    return np.float32(loss)


def kernel(all_image_features, all_text_features, logit_scale, labels):
    from concourse.bass_utils import run_bass_kernel_spmd

    img = np.ascontiguousarray(np.asarray(all_image_features), np.float32)
    txt = np.ascontiguousarray(np.asarray(all_text_features), np.float32)
    lab = np.asarray(labels)
    assert img.shape == (N, D) and txt.shape == (N, D)
    if not np.array_equal(lab.astype(np.int64), np.arange(N, dtype=np.int64)):
        return _fallback(img, txt, lab)

    img8 = _round_fp8(img)
    txt8 = _round_fp8(txt)

    # DoubleRow layout: dim g = db*2 + c maps to d = db*256 + c*128 + p.
    # moving:    X_g[p, jt, g, j] = x[jt*512 + j, d(g, p)]
    # stationary X_l[p, g, i]    = x[block_k][i, d(g, p)]
    def moving(x8):
        return np.ascontiguousarray(
            x8.reshape(JT, 512, 4, 2, P).transpose(4, 0, 2, 3, 1).reshape(P, JT, 8, 512)
        )

    def stationary(x8):
        return np.ascontiguousarray(
            x8.reshape(R, 4, 2, P).transpose(3, 1, 2, 0).reshape(P, 8, R)
        )

    textT_g = moving(txt8)
    imgT_g = moving(img8)
    in_maps = []
    for k in range(NC):
        sl = slice(k * R, (k + 1) * R)
        in_maps.append({
            "imgT_l": stationary(img8[sl]),
            "textT_l": stationary(txt8[sl]),
            "textT_g": textT_g,
            "imgT_g": imgT_g,
        })

    nc = _get_nc()
    _BUILD_CACHE["in_maps"] = in_maps
    res = run_bass_kernel_spmd(nc, in_maps, list(range(NC)))

    # ---- host-side combine (O(N) work, float64) ----
    def gather(name):
        return np.concatenate(
            [res.results[k][name].astype(np.float64).T.reshape(R) for k in range(NC)]
        )

    rA, rB = gather("out_rA"), gather("out_rB")
    dA, dB = gather("out_dA"), gather("out_dB")

    def ce_loss(r, d):
        gd = np.exp(d) - 1.0 - d - d * d / 2.0 - d ** 3 / 6.0
        lse = np.log(N + r + r * r / (2.0 * N) + r ** 3 / (6.0 * N * N) + gd)
        return np.mean(lse - d)

    return np.float32(0.5 * (ce_loss(rA, dA) + ce_loss(rB, dB)))


# revision 5
# speedup vs baseline: 1.6844x; 1.0134x over previous
"""DBOT Sinkhorn loss kernel for 8 Trainium2 NeuronCores.

Strategy (v2 — all-PE dual-slab)
--------------------------------
S = img @ text.T serves both cross-entropy terms (logits_per_text = S.T).
The Sinkhorn scalings factor as P = diag(u) P0 diag(v), P0 = exp(S-1), so
each iteration only needs matrix-vector products with P0 / P0^T.

Each core k holds TWO fp8 slabs in SBUF (64 KB/partition each):
  slabA = P0[rows Ik, :]      (row block of P0)
  slabB = P0^T[rows Ik, :]    (= column block of P0, transposed)
built by two fp8-DoubleRow gram matmuls (textT / imgT as moving operands).
With both slabs, every Sinkhorn product becomes a PE column-sum mat-vec
(contract over partitions) in fp8 DoubleRow — ~2x the bf16 rate — and the
tensor engine never idles long enough for the HAM clock-gate to rethrottle.

Each product yields an [N] partial; a ReduceScatter hands core k exactly
its local slice, which is also the slice needed for the next stationary
vector.  Stationary vectors are rescaled by their first element and cast
to fp8 (the Sinkhorn recurrence is self-correcting; validated to ~5e-6
final relative error in simulation).

The cross entropy uses that final-plan entries are small (~0.1):
  lse_i = log(N + r_i + r_i^2/2N + r_i^3/6N^2 + (e^d - 1 - d - d^2/2 - d^3/6))
with r_i = u_i (P0 v)_i (one more product) and d = u_i P0_ii v_i (diag from
features).  Host combines per-core [R] outputs in float64.
"""

import sys

sys.path.insert(0, "/opt/trn_rl_repo")

import numpy as np

N = 8192
D = 1024
NC = 8
R = N // NC          # rows per core
P = 128              # SBUF partitions
IB = R // P          # 8 row blocks per core
JT = N // 512        # 16 column tiles of 512
ITERS = 5
BD = 0.1 * N
BU = 0.9 * N
NPROD = 2 + 2 + 4 * (ITERS - 1) + 2   # products: it0 (y,w per chain) + 4/iter + final

_BUILD_CACHE = {}


def _round_fp8(x):
    from concourse import mybir

    np_f8 = mybir.dt.np(mybir.dt.float8e4)
    return np.ascontiguousarray(x, np.float32).astype(np_f8)


def _split_excess_waits(nc, max_waits=1):
    """Walrus CTRL lowering rejects instructions carrying several sem waits.
    Hoist all but the last wait of any multi-wait instruction into dedicated
    NoOps placed immediately before it on the same engine."""
    from concourse import mybir

    for f in nc.m.functions:
        for bb in f.blocks:
            insts = bb.instructions
            new_insts = []
            for inst in insts:
                si = inst.sync_info
                if si and si.on_wait and len(si.on_wait) > max_waits:
                    waits = list(si.on_wait)
                    head, tail = waits[:-max_waits], waits[-max_waits:]
                    for k, w in enumerate(head):
                        nop = mybir.InstNoOp(
                            name=f"{inst.name}-waitsplit-{k}",
                            engine=inst.engine,
                            ins=[],
                            outs=[],
                            sync_info=type(si)(on_wait=[w], on_update=[]),
                        )
                        new_insts.append(nop)
                    inst.sync_info = type(si)(
                        on_wait=tail, on_update=list(si.on_update or [])
                    )
                new_insts.append(inst)
            bb.instructions = new_insts


def _build():
    """Build the Bass module (same SPMD program for all 8 cores)."""
    from contextlib import ExitStack

    import concourse.bass as bass
    import concourse.tile as tile
    from concourse import mybir

    f32 = mybir.dt.float32
    bf16 = mybir.dt.bfloat16
    f8 = mybir.dt.float8e4
    ALU = mybir.AluOpType
    ACTF = mybir.ActivationFunctionType
    DR = mybir.MatmulPerfMode.DoubleRow
    RG = [list(range(NC))]

    nc = bass.Bass("TRN2", target_bir_lowering=False, debug=False, num_devices=NC)

    # ---- external I/O ----
    imgT_l = nc.dram_tensor("imgT_l", [P, 8, R], f8, kind="ExternalInput")
    textT_l = nc.dram_tensor("textT_l", [P, 8, R], f8, kind="ExternalInput")
    textT_g = nc.dram_tensor("textT_g", [P, JT, 8, 512], f8, kind="ExternalInput")
    imgT_g = nc.dram_tensor("imgT_g", [P, JT, 8, 512], f8, kind="ExternalInput")

    out_rA = nc.dram_tensor("out_rA", [P, IB], f32, kind="ExternalOutput")
    out_rB = nc.dram_tensor("out_rB", [P, IB], f32, kind="ExternalOutput")
    out_dA = nc.dram_tensor("out_dA", [P, IB], f32, kind="ExternalOutput")
    out_dB = nc.dram_tensor("out_dB", [P, IB], f32, kind="ExternalOutput")

    # ---- internal DRAM ----
    d0_dram = nc.dram_tensor("d0_dram", [R], f32)
    ps_in = [nc.dram_tensor(f"ps_in{t}", [N], f32) for t in range(NPROD)]
    ps_out = [nc.dram_tensor(f"ps_out{t}", [R], f32) for t in range(NPROD)]

    UMEAN = 3000.0  # ~N * mean(exp(S-1)); fp8 scales only need ~100x accuracy

    with tile.TileContext(nc) as tc, ExitStack() as ctx:
        state = ctx.enter_context(tc.tile_pool(name="state", bufs=1))
        slabA = state.tile([P, IB, JT, 512], f8)
        slabB = state.tile([P, IB, JT, 512], f8)
        statA = state.tile([P, IB, 16], f8)
        statB = state.tile([P, IB, 16], f8)
        ones16 = state.tile([P, 1], bf16)
        negone = state.tile([P, 1], f32)
        d0 = state.tile([P, IB], f32)
        uA = state.tile([P, IB], f32)
        uB = state.tile([P, IB], f32)
        vA = state.tile([P, IB], f32)
        vB = state.tile([P, IB], f32)
        yA = state.tile([P, IB], f32)
        yB = state.tile([P, IB], f32)
        wA = state.tile([P, IB], f32)
        wB = state.tile([P, IB], f32)
        c1 = state.tile([P, IB], f32)
        c2 = state.tile([P, IB], f32)
        c3 = state.tile([P, IB], f32)

        nc.vector.memset(ones16, 1.0)
        nc.vector.memset(negone, -1.0)
        nc.vector.memset(statA, 1.0)
        nc.vector.memset(statB, 1.0)
        nc.vector.memset(vA, 1.0)
        nc.vector.memset(vB, 1.0)

        # ============ feature load + diag pre-phase ============
        featp = ctx.enter_context(tc.tile_pool(name="featp", bufs=1))
        imgT_sb = featp.tile([P, 8, R], f8)
        textTl_sb = featp.tile([P, 8, R], f8)
        nc.sync.dma_start(out=imgT_sb[:], in_=imgT_l.ap())
        nc.sync.dma_start(out=textTl_sb[:], in_=textT_l.ap())

        with (
            tc.tile_pool(name="prep", bufs=1) as prep,
            tc.tile_pool(name="preps", bufs=1, space="PSUM") as preps,
        ):
            prodD = prep.tile([P, 8, R], bf16)
            nc.vector.tensor_mul(prodD[:], imgT_sb[:], textTl_sb[:])
            ps_d = preps.tile([1, 2, 512], f32)
            for h in range(2):
                for db in range(8):
                    nc.tensor.matmul(
                        ps_d[0:1, h, :],
                        ones16[:],
                        prodD[:, db, h * 512 : (h + 1) * 512],
                        start=(db == 0),
                        stop=(db == 7),
                    )
            sd = prep.tile([1, R], f32)
            nc.scalar.activation(
                sd[0:1, :], ps_d[0:1, :, :], ACTF.Exp, bias=negone[0:1, :]
            )
            nc.sync.dma_start(out=d0_dram.ap(), in_=sd[0:1, :])
        nc.gpsimd.dma_start(
            out=d0[:], in_=d0_dram.ap().rearrange("(ib p) -> p ib", p=P)
        )

        # ============ product pools (before gram pools: LIFO close order) ============
        pps = ctx.enter_context(tc.tile_pool(name="pps", bufs=2, space="PSUM"))
        stgp = ctx.enter_context(tc.tile_pool(name="stgp", bufs=2))

        # ============ gram phase: slabA = exp(S-1), slabB = exp(S.T-1) ============
        gram_ctx = ExitStack()
        mvp = gram_ctx.enter_context(tc.tile_pool(name="mvp", bufs=2))
        gps = gram_ctx.enter_context(tc.tile_pool(name="gps", bufs=3, space="PSUM"))

        def gram(stat_sb, mv_d, slab):
            for jc in range(4):
                mv = mvp.tile([P, 4, 8, 512], f8, tag="mv")
                nc.sync.dma_start(out=mv[:], in_=mv_d.ap()[:, jc * 4 : (jc + 1) * 4, :, :])
                for ib in range(8):
                    for half in range(2):
                        ps = gps.tile([P, 2, 512], f32, tag="gps")
                        for jl in range(2):
                            for db in range(4):
                                nc.tensor.matmul(
                                    ps[:, jl, :],
                                    stat_sb[:, db * 2 : db * 2 + 2, ib * P : (ib + 1) * P],
                                    mv[:, half * 2 + jl, db * 2 : db * 2 + 2, :],
                                    start=(db == 0),
                                    stop=(db == 3),
                                    perf_mode=DR,
                                )
                        nc.scalar.activation(
                            slab[:, ib, jc * 4 + half * 2 : jc * 4 + half * 2 + 2, :],
                            ps[:],
                            ACTF.Exp,
                            bias=negone[:],
                        )

        # ============ product machinery ============
        def product(slab, stat, rescale, t):
            """ps_out[t] <- RS over cores of rescale * [N]-partial of (stat . slab)."""
            for jh in range(4):
                stg = stgp.tile([1, 4, 512], f32, tag="stg")
                for jl in range(4):
                    jt = jh * 4 + jl
                    ps = pps.tile([1, 512], f32, tag="pps")
                    for q in range(4):
                        nc.tensor.matmul(
                            ps[0:1, :],
                            stat[:, 2 * q : 2 * q + 2, 0:1],
                            slab[:, 2 * q : 2 * q + 2, jt, :],
                            start=(q == 0),
                            stop=(q == 3),
                            perf_mode=DR,
                        )
                    nc.scalar.activation(
                        stg[0:1, jl, :], ps[0:1, :], ACTF.Copy, scale=float(rescale)
                    )
                nc.sync.dma_start(
                    out=ps_in[t].ap()[jh * 2048 : (jh + 1) * 2048], in_=stg[0:1, :, :]
                )
            nc.gpsimd.collective_compute(
                "ReduceScatter", ALU.add, replica_groups=RG,
                ins=[ps_in[t].ap()], outs=[ps_out[t].ap()],
            )

        def recv_y(t, y, u, stat, qscale):
            """u = 1/y; stat = f8(u * qscale)."""
            nc.sync.dma_start(out=y[:], in_=ps_out[t].ap().rearrange("(ib p) -> p ib", p=P))
            nc.vector.reciprocal(u[:], y[:])
            nc.scalar.activation(stat[:, :, 0:1], u[:], ACTF.Copy, scale=float(qscale))

        def recv_w(t, w, v, stat, qscale):
            """colstep: v *= max(BD/c,1)*min(BU/(c*f1),1), c = v.w; stat = f8(v*qscale)."""
            nc.sync.dma_start(out=w[:], in_=ps_out[t].ap().rearrange("(ib p) -> p ib", p=P))
            nc.vector.tensor_mul(c1[:], v[:], w[:])
            nc.vector.reciprocal(c2[:], c1[:])
            nc.vector.tensor_scalar(c2[:], c2[:], BD, 1.0, op0=ALU.mult, op1=ALU.max)
            nc.vector.tensor_mul(c3[:], c1[:], c2[:])
            nc.vector.tensor_mul(v[:], v[:], c2[:])
            nc.vector.reciprocal(c1[:], c3[:])
            nc.vector.tensor_scalar(c1[:], c1[:], BU, 1.0, op0=ALU.mult, op1=ALU.min)
            nc.vector.tensor_mul(v[:], v[:], c1[:])
            nc.scalar.activation(stat[:, :, 0:1], v[:], ACTF.Copy, scale=float(qscale))

        # ============ gram + Sinkhorn, pipelined ============
        # it0 product t-slots: t0 = y_B (over slabA), t1 = y_A (over slabB),
        #                      t2 = w_B (over slabB), t3 = w_A (over slabA)
        gram(imgT_sb, textT_g, slabA)
        product(slabA, statB, 1.0, 0)              # y_B partial; RS hides under gramB
        gram(textTl_sb, imgT_g, slabB)
        recv_y(0, yB, uB, statB, UMEAN)            # statB = f8(uB * UMEAN)
        product(slabB, statA, 1.0, 1)              # y_A
        product(slabB, statB, 1.0 / UMEAN, 2)      # w_B = P0 u_B
        recv_y(1, yA, uA, statA, UMEAN)
        product(slabA, statA, 1.0 / UMEAN, 3)      # w_A = P0^T u_A
        recv_w(2, wB, vB, statB, 1.0 / BD)         # vB ~ BD; statB = f8(vB/BD)
        recv_w(3, wA, vA, statA, 1.0 / BD)
        gram_ctx.close()

        t = 4
        for it in range(1, ITERS):
            sv = BD ** it          # v magnitude entering this iteration
            su = UMEAN * BD ** it  # 1/u magnitude this iteration
            tA, tB = t, t + 1
            product(slabB, statA, sv, tA)          # y_A = P0 v_A
            product(slabA, statB, sv, tB)          # y_B = P0^T v_B
            recv_y(tA, yA, uA, statA, su)
            product(slabA, statA, 1.0 / su, t + 2)  # w_A = P0^T u_A
            recv_y(tB, yB, uB, statB, su)
            product(slabB, statB, 1.0 / su, t + 3)  # w_B = P0 u_B
            recv_w(t + 2, wA, vA, statA, 1.0 / BD ** (it + 1))
            recv_w(t + 3, wB, vB, statB, 1.0 / BD ** (it + 1))
            t += 4

        # d outputs don't depend on the final products: emit early
        nc.vector.tensor_mul(c3[:], uA[:], d0[:])
        nc.vector.tensor_mul(c3[:], c3[:], vA[:])
        nc.sync.dma_start(out=out_dA.ap(), in_=c3[:])
        nc.vector.tensor_mul(c2[:], uB[:], d0[:])
        nc.vector.tensor_mul(c2[:], c2[:], vB[:])
        nc.sync.dma_start(out=out_dB.ap(), in_=c2[:])

        # final row-sum products for the cross entropy
        sv = BD ** ITERS
        tA, tB = t, t + 1
        product(slabB, statA, sv, tA)              # y6_A = P0 v_A5
        product(slabA, statB, sv, tB)              # y6_B = P0^T v_B5
        nc.sync.dma_start(out=yA[:], in_=ps_out[tA].ap().rearrange("(ib p) -> p ib", p=P))
        nc.sync.dma_start(out=yB[:], in_=ps_out[tB].ap().rearrange("(ib p) -> p ib", p=P))

        # outputs: r = u.y6
        nc.vector.tensor_mul(c1[:], uA[:], yA[:])
        nc.sync.dma_start(out=out_rA.ap(), in_=c1[:])
        nc.vector.tensor_mul(c2[:], uB[:], yB[:])
        nc.sync.dma_start(out=out_rB.ap(), in_=c2[:])

    _split_excess_waits(nc)
    return nc


def _get_nc():
    if "nc" not in _BUILD_CACHE:
        _BUILD_CACHE["nc"] = _build()
    return _BUILD_CACHE["nc"]


def _fallback(img, txt, labels):
    """Reference math on host (only for unexpected label patterns)."""
    S = img.astype(np.float64) @ txt.astype(np.float64).T

    def sink(Pin):
        n = Pin.shape[0]
        Pm = np.exp(-Pin)
        for _ in range(ITERS):
            Pm = (1.0 / Pm.sum(1))[:, None] * Pm
            Pm = Pm * np.maximum(BD / Pm.sum(0), 1.0)[None, :]
            Pm = Pm * np.minimum(BU / Pm.sum(0), 1.0)[None, :]
        return Pm

    def ce(logits, lab):
        m = logits.max(1, keepdims=True)
        lse = np.log(np.exp(logits - m).sum(1)) + m[:, 0]
        picked = logits[np.arange(logits.shape[0]), lab]
        return np.mean(lse - picked)

    lab = np.asarray(labels, np.int64)
    loss = 0.5 * (ce(sink(1.0 - S), lab) + ce(sink(1.0 - S.T), lab))
    return np.float32(loss)


def kernel(all_image_features, all_text_features, logit_scale, labels):
    from concourse.bass_utils import run_bass_kernel_spmd

    img = np.ascontiguousarray(np.asarray(all_image_features), np.float32)
    txt = np.ascontiguousarray(np.asarray(all_text_features), np.float32)
    lab = np.asarray(labels)
    assert img.shape == (N, D) and txt.shape == (N, D)
    if not np.array_equal(lab.astype(np.int64), np.arange(N, dtype=np.int64)):
        return _fallback(img, txt, lab)

    img8 = _round_fp8(img)
    txt8 = _round_fp8(txt)

    # DoubleRow layout: dim g = db*2 + c maps to d = db*256 + c*128 + p.
    # moving:    X_g[p, jt, g, j] = x[jt*512 + j, d(g, p)]
    # stationary X_l[p, g, i]    = x[block_k][i, d(g, p)]
    def moving(x8):
        return np.ascontiguousarray(
            x8.reshape(JT, 512, 4, 2, P).transpose(4, 0, 2, 3, 1).reshape(P, JT, 8, 512)
        )

    def stationary(x8):
        return np.ascontiguousarray(
            x8.reshape(R, 4, 2, P).transpose(3, 1, 2, 0).reshape(P, 8, R)
        )

    textT_g = moving(txt8)
    imgT_g = moving(img8)
    in_maps = []
    for k in range(NC):
        sl = slice(k * R, (k + 1) * R)
        in_maps.append({
            "imgT_l": stationary(img8[sl]),
            "textT_l": stationary(txt8[sl]),
            "textT_g": textT_g,
            "imgT_g": imgT_g,
        })

    nc = _get_nc()
    _BUILD_CACHE["in_maps"] = in_maps
    res = run_bass_kernel_spmd(nc, in_maps, list(range(NC)))

    # ---- host-side combine (O(N) work, float64) ----
    def gather(name):
        return np.concatenate(
            [res.results[k][name].astype(np.float64).T.reshape(R) for k in range(NC)]
        )

    rA, rB = gather("out_rA"), gather("out_rB")
    dA, dB = gather("out_dA"), gather("out_dB")

    def ce_loss(r, d):
        gd = np.exp(d) - 1.0 - d - d * d / 2.0 - d ** 3 / 6.0
        lse = np.log(N + r + r * r / (2.0 * N) + r ** 3 / (6.0 * N * N) + gd)
        return np.mean(lse - d)

    return np.float32(0.5 * (ce_loss(rA, dA) + ce_loss(rB, dB)))


# revision 7
# speedup vs baseline: 1.7952x; 1.0658x over previous
"""DBOT Sinkhorn loss kernel for 8 Trainium2 NeuronCores.

Strategy (v2 — all-PE dual-slab)
--------------------------------
S = img @ text.T serves both cross-entropy terms (logits_per_text = S.T).
The Sinkhorn scalings factor as P = diag(u) P0 diag(v), P0 = exp(S-1), so
each iteration only needs matrix-vector products with P0 / P0^T.

Each core k holds TWO fp8 slabs in SBUF (64 KB/partition each):
  slabA = P0[rows Ik, :]      (row block of P0)
  slabB = P0^T[rows Ik, :]    (= column block of P0, transposed)
built by two fp8-DoubleRow gram matmuls (textT / imgT as moving operands).
With both slabs, every Sinkhorn product becomes a PE column-sum mat-vec
(contract over partitions) in fp8 DoubleRow — ~2x the bf16 rate — and the
tensor engine never idles long enough for the HAM clock-gate to rethrottle.

Each product yields an [N] partial; a ReduceScatter hands core k exactly
its local slice, which is also the slice needed for the next stationary
vector.  Stationary vectors are rescaled by their first element and cast
to fp8 (the Sinkhorn recurrence is self-correcting; validated to ~5e-6
final relative error in simulation).

The cross entropy uses that final-plan entries are small (~0.1):
  lse_i = log(N + r_i + r_i^2/2N + r_i^3/6N^2 + (e^d - 1 - d - d^2/2 - d^3/6))
with r_i = u_i (P0 v)_i (one more product) and d = u_i P0_ii v_i (diag from
features).  Host combines per-core [R] outputs in float64.
"""

import sys

sys.path.insert(0, "/opt/trn_rl_repo")

import numpy as np

N = 8192
D = 1024
NC = 8
R = N // NC          # rows per core
P = 128              # SBUF partitions
IB = R // P          # 8 row blocks per core
JT = N // 512        # 16 column tiles of 512
ITERS = 5
BD = 0.1 * N
BU = 0.9 * N
NPROD = 2 + 2 + 4 * (ITERS - 1) + 2   # products: it0 (y,w per chain) + 4/iter + final

_BUILD_CACHE = {}


def _round_fp8(x):
    from concourse import mybir

    np_f8 = mybir.dt.np(mybir.dt.float8e4)
    return np.ascontiguousarray(x, np.float32).astype(np_f8)


def _split_excess_waits(nc, max_waits=1):
    """Walrus CTRL lowering rejects instructions carrying several sem waits.
    Hoist all but the last wait of any multi-wait instruction into dedicated
    NoOps placed immediately before it on the same engine."""
    from concourse import mybir

    for f in nc.m.functions:
        for bb in f.blocks:
            insts = bb.instructions
            new_insts = []
            for inst in insts:
                si = inst.sync_info
                if si and si.on_wait and len(si.on_wait) > max_waits:
                    waits = list(si.on_wait)
                    head, tail = waits[:-max_waits], waits[-max_waits:]
                    for k, w in enumerate(head):
                        nop = mybir.InstNoOp(
                            name=f"{inst.name}-waitsplit-{k}",
                            engine=inst.engine,
                            ins=[],
                            outs=[],
                            sync_info=type(si)(on_wait=[w], on_update=[]),
                        )
                        new_insts.append(nop)
                    inst.sync_info = type(si)(
                        on_wait=tail, on_update=list(si.on_update or [])
                    )
                new_insts.append(inst)
            bb.instructions = new_insts


def _build():
    """Build the Bass module (same SPMD program for all 8 cores)."""
    from contextlib import ExitStack

    import concourse.bass as bass
    import concourse.tile as tile
    from concourse import mybir

    f32 = mybir.dt.float32
    bf16 = mybir.dt.bfloat16
    f8 = mybir.dt.float8e4
    ALU = mybir.AluOpType
    ACTF = mybir.ActivationFunctionType
    DR = mybir.MatmulPerfMode.DoubleRow
    RG = [list(range(NC))]

    nc = bass.Bass("TRN2", target_bir_lowering=False, debug=False, num_devices=NC)

    # ---- external I/O ----
    imgT_l = nc.dram_tensor("imgT_l", [P, 8, R], f8, kind="ExternalInput")
    textT_l = nc.dram_tensor("textT_l", [P, 8, R], f8, kind="ExternalInput")
    textT_g = nc.dram_tensor("textT_g", [P, JT, 8, 512], f8, kind="ExternalInput")
    imgT_g = nc.dram_tensor("imgT_g", [P, JT, 8, 512], f8, kind="ExternalInput")

    out_rA = nc.dram_tensor("out_rA", [P, IB], f32, kind="ExternalOutput")
    out_rB = nc.dram_tensor("out_rB", [P, IB], f32, kind="ExternalOutput")
    out_dA = nc.dram_tensor("out_dA", [P, IB], f32, kind="ExternalOutput")
    out_dB = nc.dram_tensor("out_dB", [P, IB], f32, kind="ExternalOutput")

    # ---- internal DRAM ----
    d0_dram = nc.dram_tensor("d0_dram", [R], f32)
    ps_in = [nc.dram_tensor(f"ps_in{t}", [N], f32) for t in range(NPROD)]
    ps_out = [nc.dram_tensor(f"ps_out{t}", [R], f32) for t in range(NPROD)]

    UMEAN = 3000.0  # ~N * mean(exp(S-1)); fp8 scales only need ~100x accuracy

    with tile.TileContext(nc) as tc, ExitStack() as ctx:
        state = ctx.enter_context(tc.tile_pool(name="state", bufs=1))
        slabA = state.tile([P, IB, JT, 512], f8)
        slabB = state.tile([P, IB, JT, 512], f8)
        statA = state.tile([P, IB, 16], f8)
        statB = state.tile([P, IB, 16], f8)
        ones16 = state.tile([P, 1], bf16)
        negone = state.tile([P, 1], f32)
        d0 = state.tile([P, IB], f32)
        uA = state.tile([P, IB], f32)
        uB = state.tile([P, IB], f32)
        vA = state.tile([P, IB], f32)
        vB = state.tile([P, IB], f32)
        yA = state.tile([P, IB], f32)
        yB = state.tile([P, IB], f32)
        wA = state.tile([P, IB], f32)
        wB = state.tile([P, IB], f32)
        c1 = state.tile([P, IB], f32)
        c2 = state.tile([P, IB], f32)
        c3 = state.tile([P, IB], f32)

        nc.vector.memset(ones16, 1.0)
        nc.vector.memset(negone, -1.0)
        nc.vector.memset(statA, 1.0)
        nc.vector.memset(statB, 1.0)
        nc.vector.memset(vA, 1.0)
        nc.vector.memset(vB, 1.0)

        # ============ feature load + diag pre-phase ============
        featp = ctx.enter_context(tc.tile_pool(name="featp", bufs=1))
        imgT_sb = featp.tile([P, 8, R], f8)
        textTl_sb = featp.tile([P, 8, R], f8)
        nc.sync.dma_start(out=imgT_sb[:], in_=imgT_l.ap())
        nc.sync.dma_start(out=textTl_sb[:], in_=textT_l.ap())

        with (
            tc.tile_pool(name="prep", bufs=1) as prep,
            tc.tile_pool(name="preps", bufs=1, space="PSUM") as preps,
        ):
            prodD = prep.tile([P, 8, R], bf16)
            nc.vector.tensor_mul(prodD[:], imgT_sb[:], textTl_sb[:])
            ps_d = preps.tile([1, 2, 512], f32)
            for h in range(2):
                for db in range(8):
                    nc.tensor.matmul(
                        ps_d[0:1, h, :],
                        ones16[:],
                        prodD[:, db, h * 512 : (h + 1) * 512],
                        start=(db == 0),
                        stop=(db == 7),
                    )
            sd = prep.tile([1, R], f32)
            nc.scalar.activation(
                sd[0:1, :], ps_d[0:1, :, :], ACTF.Exp, bias=negone[0:1, :]
            )
            nc.sync.dma_start(out=d0_dram.ap(), in_=sd[0:1, :])
        nc.gpsimd.dma_start(
            out=d0[:], in_=d0_dram.ap().rearrange("(ib p) -> p ib", p=P)
        )

        # ============ product pools (before gram pools: LIFO close order) ============
        pps = ctx.enter_context(tc.tile_pool(name="pps", bufs=2, space="PSUM"))
        stgp = ctx.enter_context(tc.tile_pool(name="stgp", bufs=2))

        # ============ gram phase: slabA = exp(S-1), slabB = exp(S.T-1) ============
        gram_ctx = ExitStack()
        mvp = gram_ctx.enter_context(tc.tile_pool(name="mvp", bufs=2))
        gps = gram_ctx.enter_context(tc.tile_pool(name="gps", bufs=3, space="PSUM"))

        def gram(stat_sb, mv_d, slab):
            for jc in range(4):
                mv = mvp.tile([P, 4, 8, 512], f8, tag="mv")
                nc.sync.dma_start(out=mv[:], in_=mv_d.ap()[:, jc * 4 : (jc + 1) * 4, :, :])
                for ib in range(8):
                    for half in range(2):
                        ps = gps.tile([P, 2, 512], f32, tag="gps")
                        for jl in range(2):
                            for db in range(4):
                                nc.tensor.matmul(
                                    ps[:, jl, :],
                                    stat_sb[:, db * 2 : db * 2 + 2, ib * P : (ib + 1) * P],
                                    mv[:, half * 2 + jl, db * 2 : db * 2 + 2, :],
                                    start=(db == 0),
                                    stop=(db == 3),
                                    perf_mode=DR,
                                )
                        nc.scalar.activation(
                            slab[:, ib, jc * 4 + half * 2 : jc * 4 + half * 2 + 2, :],
                            ps[:],
                            ACTF.Exp,
                            bias=negone[:],
                        )

        # ============ product machinery ============
        def product(slab, stat, rescale, t):
            """ps_out[t] <- RS over cores of rescale * [N]-partial of (stat . slab)."""
            for jh in range(4):
                stg = stgp.tile([1, 4, 512], f32, tag="stg")
                for jl in range(4):
                    jt = jh * 4 + jl
                    ps = pps.tile([1, 512], f32, tag="pps")
                    for q in range(4):
                        nc.tensor.matmul(
                            ps[0:1, :],
                            stat[:, 2 * q : 2 * q + 2, 0:1],
                            slab[:, 2 * q : 2 * q + 2, jt, :],
                            start=(q == 0),
                            stop=(q == 3),
                            perf_mode=DR,
                        )
                    nc.scalar.activation(
                        stg[0:1, jl, :], ps[0:1, :], ACTF.Copy, scale=float(rescale)
                    )
                nc.sync.dma_start(
                    out=ps_in[t].ap()[jh * 2048 : (jh + 1) * 2048], in_=stg[0:1, :, :]
                )
            nc.gpsimd.collective_compute(
                "ReduceScatter", ALU.add, replica_groups=RG,
                ins=[ps_in[t].ap()], outs=[ps_out[t].ap()],
            )

        def recv_y(t, y, u, stat, qscale):
            """u = 1/y; stat = f8(u * qscale)."""
            nc.gpsimd.dma_start(out=y[:], in_=ps_out[t].ap().rearrange("(ib p) -> p ib", p=P))
            nc.vector.reciprocal(u[:], y[:])
            nc.vector.tensor_scalar_mul(stat[:, :, 0:1], u[:], float(qscale))

        def recv_w(t, w, v, stat, qscale):
            """colstep: v *= max(BD/c,1)*min(BU/(c*f1),1), c = v.w; stat = f8(v*qscale)."""
            nc.gpsimd.dma_start(out=w[:], in_=ps_out[t].ap().rearrange("(ib p) -> p ib", p=P))
            nc.vector.tensor_mul(c1[:], v[:], w[:])
            nc.vector.reciprocal(c2[:], c1[:])
            nc.vector.tensor_scalar(c2[:], c2[:], BD, 1.0, op0=ALU.mult, op1=ALU.max)
            nc.vector.tensor_mul(c3[:], c1[:], c2[:])
            nc.vector.tensor_mul(v[:], v[:], c2[:])
            nc.vector.reciprocal(c1[:], c3[:])
            nc.vector.tensor_scalar(c1[:], c1[:], BU, 1.0, op0=ALU.mult, op1=ALU.min)
            nc.vector.tensor_mul(v[:], v[:], c1[:])
            nc.vector.tensor_scalar_mul(stat[:, :, 0:1], v[:], float(qscale))

        # ============ gram + Sinkhorn, pipelined ============
        # it0 product t-slots: t0 = y_B (over slabA), t1 = y_A (over slabB),
        #                      t2 = w_B (over slabB), t3 = w_A (over slabA)
        gram(imgT_sb, textT_g, slabA)
        product(slabA, statB, 1.0, 0)              # y_B partial; RS hides under gramB
        gram(textTl_sb, imgT_g, slabB)
        recv_y(0, yB, uB, statB, UMEAN)            # statB = f8(uB * UMEAN)
        product(slabB, statA, 1.0, 1)              # y_A
        product(slabB, statB, 1.0 / UMEAN, 2)      # w_B = P0 u_B
        recv_y(1, yA, uA, statA, UMEAN)
        product(slabA, statA, 1.0 / UMEAN, 3)      # w_A = P0^T u_A
        recv_w(2, wB, vB, statB, 1.0 / BD)         # vB ~ BD; statB = f8(vB/BD)
        recv_w(3, wA, vA, statA, 1.0 / BD)
        gram_ctx.close()

        t = 4
        for it in range(1, ITERS):
            sv = BD ** it          # v magnitude entering this iteration
            su = UMEAN * BD ** it  # 1/u magnitude this iteration
            tA, tB = t, t + 1
            product(slabB, statA, sv, tA)          # y_A = P0 v_A
            product(slabA, statB, sv, tB)          # y_B = P0^T v_B
            recv_y(tA, yA, uA, statA, su)
            product(slabA, statA, 1.0 / su, t + 2)  # w_A = P0^T u_A
            recv_y(tB, yB, uB, statB, su)
            product(slabB, statB, 1.0 / su, t + 3)  # w_B = P0 u_B
            recv_w(t + 2, wA, vA, statA, 1.0 / BD ** (it + 1))
            recv_w(t + 3, wB, vB, statB, 1.0 / BD ** (it + 1))
            t += 4

        # d outputs don't depend on the final products: emit early
        nc.vector.tensor_mul(c3[:], uA[:], d0[:])
        nc.vector.tensor_mul(c3[:], c3[:], vA[:])
        nc.sync.dma_start(out=out_dA.ap(), in_=c3[:])
        nc.vector.tensor_mul(c2[:], uB[:], d0[:])
        nc.vector.tensor_mul(c2[:], c2[:], vB[:])
        nc.sync.dma_start(out=out_dB.ap(), in_=c2[:])

        # final row-sum products for the cross entropy
        sv = BD ** ITERS
        tA, tB = t, t + 1
        product(slabB, statA, sv, tA)              # y6_A = P0 v_A5
        product(slabA, statB, sv, tB)              # y6_B = P0^T v_B5
        nc.sync.dma_start(out=yA[:], in_=ps_out[tA].ap().rearrange("(ib p) -> p ib", p=P))
        nc.sync.dma_start(out=yB[:], in_=ps_out[tB].ap().rearrange("(ib p) -> p ib", p=P))

        # outputs: r = u.y6
        nc.vector.tensor_mul(c1[:], uA[:], yA[:])
        nc.sync.dma_start(out=out_rA.ap(), in_=c1[:])
        nc.vector.tensor_mul(c2[:], uB[:], yB[:])
        nc.sync.dma_start(out=out_rB.ap(), in_=c2[:])

    _split_excess_waits(nc)
    return nc


def _get_nc():
    if "nc" not in _BUILD_CACHE:
        _BUILD_CACHE["nc"] = _build()
    return _BUILD_CACHE["nc"]


def _fallback(img, txt, labels):
    """Reference math on host (only for unexpected label patterns)."""
    S = img.astype(np.float64) @ txt.astype(np.float64).T

    def sink(Pin):
        n = Pin.shape[0]
        Pm = np.exp(-Pin)
        for _ in range(ITERS):
            Pm = (1.0 / Pm.sum(1))[:, None] * Pm
            Pm = Pm * np.maximum(BD / Pm.sum(0), 1.0)[None, :]
            Pm = Pm * np.minimum(BU / Pm.sum(0), 1.0)[None, :]
        return Pm

    def ce(logits, lab):
        m = logits.max(1, keepdims=True)
        lse = np.log(np.exp(logits - m).sum(1)) + m[:, 0]
        picked = logits[np.arange(logits.shape[0]), lab]
        return np.mean(lse - picked)

    lab = np.asarray(labels, np.int64)
    loss = 0.5 * (ce(sink(1.0 - S), lab) + ce(sink(1.0 - S.T), lab))
    return np.float32(loss)


def kernel(all_image_features, all_text_features, logit_scale, labels):
    from concourse.bass_utils import run_bass_kernel_spmd

    img = np.ascontiguousarray(np.asarray(all_image_features), np.float32)
    txt = np.ascontiguousarray(np.asarray(all_text_features), np.float32)
    lab = np.asarray(labels)
    assert img.shape == (N, D) and txt.shape == (N, D)
    if not np.array_equal(lab.astype(np.int64), np.arange(N, dtype=np.int64)):
        return _fallback(img, txt, lab)

    img8 = _round_fp8(img)
    txt8 = _round_fp8(txt)

    # DoubleRow layout: dim g = db*2 + c maps to d = db*256 + c*128 + p.
    # moving:    X_g[p, jt, g, j] = x[jt*512 + j, d(g, p)]
    # stationary X_l[p, g, i]    = x[block_k][i, d(g, p)]
    def moving(x8):
        return np.ascontiguousarray(
            x8.reshape(JT, 512, 4, 2, P).transpose(4, 0, 2, 3, 1).reshape(P, JT, 8, 512)
        )

    def stationary(x8):
        return np.ascontiguousarray(
            x8.reshape(R, 4, 2, P).transpose(3, 1, 2, 0).reshape(P, 8, R)
        )

    textT_g = moving(txt8)
    imgT_g = moving(img8)
    in_maps = []
    for k in range(NC):
        sl = slice(k * R, (k + 1) * R)
        in_maps.append({
            "imgT_l": stationary(img8[sl]),
            "textT_l": stationary(txt8[sl]),
            "textT_g": textT_g,
            "imgT_g": imgT_g,
        })

    nc = _get_nc()
    _BUILD_CACHE["in_maps"] = in_maps
    res = run_bass_kernel_spmd(nc, in_maps, list(range(NC)))

    # ---- host-side combine (O(N) work, float64) ----
    def gather(name):
        return np.concatenate(
            [res.results[k][name].astype(np.float64).T.reshape(R) for k in range(NC)]
        )

    rA, rB = gather("out_rA"), gather("out_rB")
    dA, dB = gather("out_dA"), gather("out_dB")

    def ce_loss(r, d):
        gd = np.exp(d) - 1.0 - d - d * d / 2.0 - d ** 3 / 6.0
        lse = np.log(N + r + r * r / (2.0 * N) + r ** 3 / (6.0 * N * N) + gd)
        return np.mean(lse - d)

    return np.float32(0.5 * (ce_loss(rA, dA) + ce_loss(rB, dB)))
